# revision 24
# baseline (speedup 1.0000x reference)
"""2-layer GATv2 over 50k nodes / 1.6M edges on 8 trn2 NeuronCores.

Strategy (self-contained; shapes hardcoded for this problem):
  - Node-parallel dst sharding: nodes are degree-sorted and dealt round-robin
    to 8 cores (balanced slot counts); each core owns 6272 dst nodes.
  - Per-shard GEMMs only (no redundant full-table work); the per-shard xl
    tables are AllGathered on-device so each core can gather arbitrary
    source rows. Host->device traffic is just the core's own x shard (fp16),
    packed u16 slot tables, and small weights.
  - Per dst node, incoming edges live in up to D=64 "slots" (max degree 61);
    per-128-node-tile slot count Dt comes from the degree sort, cutting padded
    work from 64 to ~avg-degree slots. Slot tables ship as packed u16
    [128, sum(Dt)] and are widened to i32 on-device; the -1e30 pad-logit mask
    is derived on-device from idx >= NP.
  - att is folded into the weights on the host (u = att*z); leaky-relu logits
    are computed as sum_pos relu(u) - sum_neg relu(-u) via ACT with a host
    sign-permutation of feature columns; messages are recovered from u via a
    1/att columnwise multiply (exact up to fp rounding).
  - Gather of xl rows via indirect DMA (gpsimd SWDGE) with CCE add of the
    prefilled xr broadcast; pad slots masked via the on-device -1e30 offsets.
  - Layer-1 output h is transposed on PE into a resident fp16 SBUF tile and
    used directly as lhsT for the layer-2 per-shard GEMMs.
"""
import os
os.environ.setdefault("JAX_PLATFORMS", "cpu")
import sys
if "/opt/trn_rl_repo" not in sys.path:
    sys.path.insert(0, "/opt/trn_rl_repo")
import numpy as np
import concourse.bass as bass
import concourse.bacc as bacc
import concourse.mybir as mybir
import concourse.tile as tile
from concourse import bass_utils
from concourse.masks import make_identity

f32 = mybir.dt.float32
f16 = mybir.dt.float16
i32 = mybir.dt.int32
u16 = mybir.dt.uint16
AX = mybir.AxisListType
OP = mybir.AluOpType
AF = mybir.ActivationFunctionType

N = 50000
NCORES = 8
NP = 50176          # 8 * 6272, multiple of 1024
SH = NP // NCORES   # 6272 = 49 * 128
TPS = SH // 128     # 49 tiles per shard
F_IN = 256
H = 128
C = 64
DMAX = 64
NEG = 0.2
EPS = 1e-16

GATHER_MODE = "per_d"   # "per_d" | "multi"
VARIANT = "full"        # "full" | "nogather" | "noedge" (timing ablations)
BCAST_ADD = True        # single DVE broadcast add of xr (False: per-slot adds)
EDGE_SKIP = ()          # timing ablations: subset of
                        # {"acc", "logit", "exp", "trans"}

LAST_RESULT = None
LAST_RUN_WALL = None
_PROGRAM_CACHE = {}


def ts(i, s):
    return slice(i * s, (i + 1) * s)


def ceil4(v):
    return max(4, (int(v) + 3) // 4 * 4)


def build_program(Dts, Fp1, Fp2):
    key = (tuple(Dts), Fp1, Fp2, GATHER_MODE, VARIANT, BCAST_ADD,
           tuple(EDGE_SKIP))
    if key in _PROGRAM_CACHE:
        return _PROGRAM_CACHE[key]
    TOTD = sum(Dts)
    cums = [0]
    for d in Dts:
        cums.append(cums[-1] + d)

    nc = bacc.Bacc("TRN2", target_bir_lowering=False, debug=False,
                   enable_asserts=False, num_devices=NCORES)

    xTo = nc.dram_tensor("xTo", [F_IN, SH], f16, kind="ExternalInput")
    wl1 = nc.dram_tensor("wl1", [F_IN, H + 1], f16, kind="ExternalInput")
    wr1 = nc.dram_tensor("wr1", [F_IN, H + 1], f16, kind="ExternalInput")
    wl2 = nc.dram_tensor("wl2", [H, C + 1], f16, kind="ExternalInput")
    wr2 = nc.dram_tensor("wr2", [H, C + 1], f16, kind="ExternalInput")
    slotp = nc.dram_tensor("slotp", [128, TOTD], u16, kind="ExternalInput")
    rc1 = nc.dram_tensor("rc1", [128, H], f32, kind="ExternalInput")
    cb1 = nc.dram_tensor("cb1", [128, H], f32, kind="ExternalInput")
    rc2 = nc.dram_tensor("rc2", [128, C], f32, kind="ExternalInput")
    cb2 = nc.dram_tensor("cb2", [128, C], f32, kind="ExternalInput")
    outc = nc.dram_tensor("outc", [SH, C], f16, kind="ExternalOutput")

    xl1t = nc.dram_tensor("xl1t", [SH, H + 1], f16, kind="Internal")
    xl1f = nc.dram_tensor("xl1f", [NP, H + 1], f16, kind="Internal",
                          addr_space="Shared")
    xl2t = nc.dram_tensor("xl2t", [SH, C + 1], f16, kind="Internal")
    xl2f = nc.dram_tensor("xl2f", [NP, C + 1], f16, kind="Internal",
                          addr_space="Shared")

    with tile.TileContext(nc) as tc:
        with tc.tile_pool(name="pers", bufs=1) as pers:
            # persistent SBUF residents (span all phases)
            xr1_sb = pers.tile([128, TPS, H + 1], f16)
            hT_sb = pers.tile([128, SH], f16)
            xr2_sb = pers.tile([128, TPS, C + 1], f16)
            slot_i = pers.tile([128, TOTD], i32)
            moff_sb = pers.tile([128, TOTD], f32)
            rc1_t = pers.tile([128, H], f32)
            cb1_t = pers.tile([128, H], f32)
            rc2_t = pers.tile([128, C], f32)
            cb2_t = pers.tile([128, C], f32)
            ident = pers.tile([128, 128], f32)
            wl2_t = pers.tile([128, C + 1], f16)
            wr2_t = pers.tile([128, C + 1], f16)

            make_identity(nc, ident[:])
            nc.sync.dma_start(out=rc1_t[:], in_=rc1.ap())
            nc.sync.dma_start(out=cb1_t[:], in_=cb1.ap())
            nc.sync.dma_start(out=rc2_t[:], in_=rc2.ap())
            nc.sync.dma_start(out=cb2_t[:], in_=cb2.ap())
            nc.sync.dma_start(out=wl2_t[:], in_=wl2.ap())
            nc.sync.dma_start(out=wr2_t[:], in_=wr2.ap())

            # slot table: u16 load -> f32; mask from idx>=NP; clamp for the
            # gather (pads read row NP-1, masked to alpha=0 by moff)
            with tc.tile_pool(name="pslot", bufs=1) as pslot:
                slot_u = pslot.tile([128, TOTD], u16)
                nc.sync.dma_start(out=slot_u[:], in_=slotp.ap())
                slot_f = pslot.tile([128, TOTD], f32)
                nc.vector.tensor_copy(out=slot_f[:], in_=slot_u[:])
                nc.vector.tensor_scalar(out=moff_sb[:], in0=slot_f[:],
                                        scalar1=float(NP) - 0.5,
                                        scalar2=-1e30,
                                        op0=OP.is_ge, op1=OP.mult)
                nc.vector.tensor_scalar_min(slot_f[:], slot_f[:],
                                            float(NP - 1))
                nc.vector.tensor_copy(out=slot_i[:], in_=slot_f[:])

            # ---------------- Phase A: layer-1 GEMMs (own shard) ----------
            with (
                tc.tile_pool(name="paw", bufs=1) as pw,
                tc.tile_pool(name="pa", bufs=4) as pa,
                tc.tile_pool(name="pap", bufs=4, space="PSUM") as pp,
            ):
                wl_t = pw.tile([128, 2, H + 1], f16)
                wr_t = pw.tile([128, 2, H + 1], f16)
                for k in range(2):
                    nc.sync.dma_start(out=wl_t[:, k, :],
                                      in_=wl1.ap()[ts(k, 128), :])
                    nc.sync.dma_start(out=wr_t[:, k, :],
                                      in_=wr1.ap()[ts(k, 128), :])
                for t in range(TPS):
                    xt_t = pa.tile([128, 2, 128], f16, tag="xt")
                    for k in range(2):
                        nc.sync.dma_start(out=xt_t[:, k, :],
                                          in_=xTo.ap()[ts(k, 128), ts(t, 128)])
                    psl_t = pp.tile([128, H + 1], f32, tag="psl")
                    for k in range(2):
                        nc.tensor.matmul(out=psl_t[:], lhsT=xt_t[:, k, :],
                                         rhs=wl_t[:, k, :],
                                         start=(k == 0), stop=(k == 1))
                    o_t = pa.tile([128, H + 1], f16, tag="o")
                    nc.scalar.copy(out=o_t[:], in_=psl_t[:])
                    nc.sync.dma_start(out=xl1t.ap()[ts(t, 128), :], in_=o_t[:])
                    psr_t = pp.tile([128, H + 1], f32, tag="psr")
                    for k in range(2):
                        nc.tensor.matmul(out=psr_t[:], lhsT=xt_t[:, k, :],
                                         rhs=wr_t[:, k, :],
                                         start=(k == 0), stop=(k == 1))
                    nc.scalar.copy(out=xr1_sb[:, t, :], in_=psr_t[:])

            # ---------------- AllGather layer-1 xl table ------------------
            nc.gpsimd.collective_compute(
                "AllGather", OP.bypass,
                replica_groups=[list(range(NCORES))],
                ins=[xl1t.ap()], outs=[xl1f.ap()])

            # ---------------- layer-1 edge phase --------------------------
            if VARIANT != "noedge":
                edge_phase(nc, tc, Dts, cums, Fp1, H, xl1f, xr1_sb, slot_i,
                           moff_sb, rc1_t, cb1_t, relu=True, out_dram=None,
                           hT_sb=hT_sb, ident=ident)
            else:
                nc.vector.memset(hT_sb[:], 0.0)

            # ---------------- Phase C: layer-2 GEMMs (own shard) ----------
            with (
                tc.tile_pool(name="pd", bufs=4) as pd,
                tc.tile_pool(name="pdp", bufs=4, space="PSUM") as pp2,
            ):
                for t in range(TPS):
                    ps2_t = pp2.tile([128, C + 1], f32, tag="ps2l")
                    nc.tensor.matmul(out=ps2_t[:], lhsT=hT_sb[:, ts(t, 128)],
                                     rhs=wl2_t[:], start=True, stop=True)
                    o2_t = pd.tile([128, C + 1], f16, tag="o2")
                    nc.scalar.copy(out=o2_t[:], in_=ps2_t[:])
                    nc.sync.dma_start(out=xl2t.ap()[ts(t, 128), :], in_=o2_t[:])
                    ps2r_t = pp2.tile([128, C + 1], f32, tag="ps2r")
                    nc.tensor.matmul(out=ps2r_t[:], lhsT=hT_sb[:, ts(t, 128)],
                                     rhs=wr2_t[:], start=True, stop=True)
                    nc.scalar.copy(out=xr2_sb[:, t, :], in_=ps2r_t[:])

            # ---------------- AllGather layer-2 xl table ------------------
            nc.gpsimd.collective_compute(
                "AllGather", OP.bypass,
                replica_groups=[list(range(NCORES))],
                ins=[xl2t.ap()], outs=[xl2f.ap()])

            # ---------------- layer-2 edge phase --------------------------
            if VARIANT != "noedge":
                edge_phase(nc, tc, Dts, cums, Fp2, C, xl2f, xr2_sb, slot_i,
                           moff_sb, rc2_t, cb2_t, relu=False, out_dram=outc,
                           hT_sb=None, ident=None)
            else:
                with tc.tile_pool(name="pdum", bufs=2) as pdum:
                    for t in range(TPS):
                        dt_ = pdum.tile([128, C], f16, tag="d")
                        nc.vector.tensor_copy(out=dt_[:],
                                              in_=xr2_sb[:, t, 0:C])
                        nc.sync.dma_start(out=outc.ap()[ts(t, 128), :],
                                          in_=dt_[:])

    nc.compile()
    _PROGRAM_CACHE[key] = nc
    return nc


def edge_phase(nc, tc, Dts, cums, Fp, F, xlf, xr_sb, slot_i, moff_sb,
               rc_t, cb_t, relu, out_dram, hT_sb, ident):
    TW = F + 1   # table width: F features + q (= row-sum) column
    with (
        tc.tile_pool(name=f"pz{F}", bufs=2) as pz,
        tc.tile_pool(name=f"pw{F}", bufs=2) as pwv,
        tc.tile_pool(name=f"pm{F}", bufs=3) as psm,
        tc.tile_pool(name=f"po{F}", bufs=2) as pout,
        tc.tile_pool(name=f"pp{F}", bufs=2, space="PSUM") as pps,
    ):
        for t in range(TPS):
            Dt = Dts[t]
            cu = cums[t]
            idx_t = slot_i[:, cu:cu + Dt]
            off_t = moff_sb[:, cu:cu + Dt]
            xr_t = xr_sb[:, t, :]

            # z = gathered xl rows (bypass; pad idx clamped to NP-1), then
            # u = z + xr via in-place broadcast add.
            z_t = pz.tile([128, Dt, TW], f16, tag="z")
            if VARIANT == "nogather":
                nc.vector.tensor_copy(
                    out=z_t[:],
                    in_=xr_t[:, None, :].to_broadcast([128, Dt, TW]))
            elif GATHER_MODE == "multi":
                nc.gpsimd.indirect_dma_start(
                    out=z_t[:], out_offset=None, in_=xlf.ap(),
                    in_offset=bass.IndirectOffsetOnAxis(ap=idx_t, axis=0),
                    bounds_check=None, compute_op=OP.bypass)
            else:
                for d in range(Dt):
                    nc.gpsimd.indirect_dma_start(
                        out=z_t[:, d, :], out_offset=None, in_=xlf.ap(),
                        in_offset=bass.IndirectOffsetOnAxis(
                            ap=idx_t[:, d:d + 1], axis=0),
                        bounds_check=None, compute_op=OP.bypass)
            if BCAST_ADD:
                nc.vector.tensor_tensor(
                    out=z_t[:], in0=z_t[:],
                    in1=xr_t[:, None, :].to_broadcast([128, Dt, TW]),
                    op=OP.add)
            else:
                for d in range(Dt):
                    nc.vector.tensor_tensor(out=z_t[:, d, :],
                                            in0=z_t[:, d, :],
                                            in1=xr_t, op=OP.add)

            # logits. With u = xl+xr (in z) and the q column sigma = sum_f u:
            #   lrelu identity: 0.8*(sum_pos relu(u) - sum_neg relu(-u))
            #     + 0.2*sigma = 0.6*sigma + 0.4*(sum_pos |u| - sum_neg |u|)
            # so two abs-reduces replace the ACT relus entirely.
            e_t = psm.tile([128, Dt], f32, tag="e")
            if "logit" in EDGE_SKIP:
                nc.vector.tensor_copy(out=e_t[:], in_=off_t)
            else:
                ep_t = psm.tile([128, Dt], f32, tag="ep")
                nc.vector.tensor_reduce(out=ep_t[:], in_=z_t[:, :, 0:Fp],
                                        axis=AX.X, op=OP.add,
                                        apply_absolute_value=True)
                en_t = psm.tile([128, Dt], f32, tag="en")
                nc.vector.tensor_reduce(out=en_t[:], in_=z_t[:, :, Fp:F],
                                        axis=AX.X, op=OP.add,
                                        apply_absolute_value=True)
                nc.vector.scalar_tensor_tensor(out=e_t[:], in0=en_t[:],
                                               scalar=-1.0, in1=ep_t[:],
                                               op0=OP.mult, op1=OP.add)
                nc.vector.scalar_tensor_tensor(out=e_t[:], in0=z_t[:, :, F],
                                               scalar=1.5, in1=e_t[:],
                                               op0=OP.mult, op1=OP.add)
                nc.vector.scalar_tensor_tensor(out=e_t[:], in0=e_t[:],
                                               scalar=0.4, in1=off_t,
                                               op0=OP.mult, op1=OP.add)
            mneg_t = psm.tile([128, 1], f32, tag="mneg")
            nc.vector.tensor_reduce(out=mneg_t[:], in_=e_t[:], axis=AX.X,
                                    op=OP.max, negate=True)
            nc.vector.tensor_scalar_min(mneg_t[:], mneg_t[:], 1e29)
            a_t = psm.tile([128, Dt], f32, tag="a")
            if "exp" in EDGE_SKIP:
                nc.vector.tensor_scalar_add(a_t[:], e_t[:], 1e-3)
            else:
                nc.scalar.activation(out=a_t[:], in_=e_t[:], func=AF.Exp,
                                     bias=mneg_t[:, :1])
            s_t = psm.tile([128, 1], f32, tag="s")
            nc.vector.tensor_reduce(out=s_t[:], in_=a_t[:], axis=AX.X,
                                    op=OP.add)
            nc.vector.tensor_scalar_add(s_t[:], s_t[:], EPS)
            r_t = psm.tile([128, 1], f32, tag="r")
            nc.vector.reciprocal(out=r_t[:], in_=s_t[:])
            al_t = psm.tile([128, Dt], f32, tag="al")
            nc.vector.tensor_scalar_mul(al_t[:], a_t[:], r_t[:, :1])

            # message aggregation: msg = sum_d alpha_d * u_d - (sum alpha)*xr
            # (z holds u = xr+g). One broadcast multiply + one reduce over
            # the slot axis via a transposed AP view.
            wz_t = pwv.tile([128, Dt, F], f16, tag="wz")
            nc.vector.tensor_tensor(
                out=wz_t[:], in0=z_t[:, :, 0:F],
                in1=al_t[:, :, None].to_broadcast([128, Dt, F]), op=OP.mult)
            acc_t = pout.tile([128, F], f32, tag="acc")
            nc.vector.tensor_reduce(out=acc_t[:],
                                    in_=wz_t[:].transpose([0, 2, 1]),
                                    axis=AX.X, op=OP.add)
            hh_t = pout.tile([128, F], f32, tag="hh")
            saneg_t = psm.tile([128, 1], f32, tag="saneg")
            nc.vector.tensor_reduce(out=saneg_t[:], in_=al_t[:],
                                    axis=AX.X, op=OP.add, negate=True)
            nc.vector.scalar_tensor_tensor(
                out=hh_t[:], in0=xr_t[:, 0:F], scalar=saneg_t[:, :1],
                in1=acc_t[:], op0=OP.mult, op1=OP.add)
            nc.vector.tensor_tensor(out=hh_t[:], in0=hh_t[:],
                                    in1=rc_t[:], op=OP.mult)
            if relu:
                nc.vector.tensor_tensor(out=hh_t[:], in0=hh_t[:], in1=cb_t[:],
                                        op=OP.add)
                nc.vector.tensor_scalar_max(hh_t[:], hh_t[:], 0.0)
                if "trans" in EDGE_SKIP:
                    if t == 0:
                        nc.vector.memset(hT_sb[:], 0.0)
                else:
                    pt_t = pps.tile([128, 128], f32, tag="pt")
                    nc.tensor.transpose(out=pt_t[:], in_=hh_t[:],
                                        identity=ident[:])
                    nc.scalar.copy(out=hT_sb[:, ts(t, 128)], in_=pt_t[:])
            else:
                ho_t = pout.tile([128, F], f16, tag="ho")
                nc.vector.tensor_tensor(out=ho_t[:], in0=hh_t[:], in1=cb_t[:],
                                        op=OP.add)
                nc.sync.dma_start(out=out_dram.ap()[ts(t, 128), :],
                                  in_=ho_t[:])


def prepare_host(x, edge_index, Wl1, Wr1, att1, b1, Wl2, Wr2, att2, b2):
    src = np.asarray(edge_index[0], dtype=np.int64)
    dst = np.asarray(edge_index[1], dtype=np.int64)
    x = np.asarray(x, dtype=np.float32)

    deg = np.bincount(dst, minlength=NP).astype(np.int64)
    assert deg.max() <= DMAX, f"max degree {deg.max()} > {DMAX}"
    order = np.argsort(-deg, kind="stable")
    q = np.arange(NP)
    new_of = np.empty(NP, dtype=np.int64)
    new_of[order] = (q % NCORES) * SH + q // NCORES
    glob_of_new = np.empty(NP, dtype=np.int64)
    glob_of_new[new_of] = np.arange(NP)

    # slot tables (values are NEW ids; rows ordered by NEW id)
    eorder = np.argsort(dst, kind="stable")
    s_src = src[eorder]
    s_dst = dst[eorder]
    starts = np.zeros(NP, dtype=np.int64)
    starts[1:] = np.cumsum(deg)[:-1]
    pos = np.arange(len(s_dst)) - starts[s_dst]
    # pads point at idx=NP: skipped by the gather bounds check, masked by the
    # on-device idx>=NP -> -1e30 logit offset
    slot_g = np.full((NP, DMAX), NP, dtype=np.int32)
    slot_g[s_dst, pos] = new_of[s_src].astype(np.int32)
    slot_new = slot_g[glob_of_new]

    deg_sorted = deg[order]
    Dts = tuple(ceil4(max(deg_sorted[1024 * t], 1)) for t in range(TPS))

    att1 = np.asarray(att1, np.float32)
    att2 = np.asarray(att2, np.float32)
    assert np.abs(att1).min() > 1e-8 and np.abs(att2).min() > 1e-8
    p1 = np.argsort(att1 < 0, kind="stable")
    Fp1 = int((att1 >= 0).sum())
    p2 = np.argsort(att2 < 0, kind="stable")
    Fp2 = int((att2 >= 0).sum())
    # fold att into weight columns, sign-permute, and append a row-sum
    # column (the q/sigma channel: sum_f u = x @ wsum)
    def fold(W, att, perm, rowperm=None):
        Wa = (np.asarray(W, np.float32) * att)
        if rowperm is not None:
            Wa = Wa[rowperm, :]
        Wp = Wa[:, perm]
        return np.ascontiguousarray(
            np.concatenate([Wp, Wp.sum(1, keepdims=True)], axis=1),
            np.float16)

    Wl1a = fold(Wl1, att1, p1)
    Wr1a = fold(Wr1, att1, p1)
    Wl2a = fold(Wl2, att2, p2, rowperm=p1)
    Wr2a = fold(Wr2, att2, p2, rowperm=p1)
    rc1_row = (1.0 / att1[p1]).astype(np.float32)
    rc2_row = (1.0 / att2[p2]).astype(np.float32)
    b1_row = np.asarray(b1, np.float32)[p1]
    b2_row = np.asarray(b2, np.float32)[p2]

    xp = np.zeros((NP, F_IN), np.float32)
    xp[:N] = x
    xT_perm = xp[glob_of_new].T.astype(np.float16)

    rep = lambda row: np.ascontiguousarray(
        np.tile(row[None, :], (128, 1)).astype(np.float32))
    common = dict(
        wl1=Wl1a, wr1=Wr1a, wl2=Wl2a, wr2=Wr2a,
        rc1=rep(rc1_row), cb1=rep(b1_row), rc2=rep(rc2_row), cb2=rep(b2_row))
    in_maps = []
    for c in range(NCORES):
        m = dict(common)
        m["xTo"] = np.ascontiguousarray(xT_perm[:, ts(c, SH)])
        sl = slot_new[ts(c, SH)]
        m["slotp"] = np.ascontiguousarray(np.concatenate(
            [sl[ts(t, 128), 0:Dts[t]] for t in range(TPS)],
            axis=1).astype(np.uint16))
        in_maps.append(m)
    return in_maps, Dts, Fp1, Fp2, glob_of_new, p2


def kernel(**inputs):
    global LAST_RESULT, LAST_RUN_WALL
    import time as _time
    in_maps, Dts, Fp1, Fp2, glob_of_new, p2 = prepare_host(**inputs)
    nc = build_program(Dts, Fp1, Fp2)
    _t0 = _time.time()
    res = bass_utils.run_bass_kernel_spmd(nc, in_maps,
                                          core_ids=list(range(NCORES)))
    LAST_RUN_WALL = _time.time() - _t0
    LAST_RESULT = res
    out_new = np.concatenate([res.results[c]["outc"] for c in range(NCORES)],
                             axis=0).astype(np.float32)
    out_glob = np.empty((NP, C), np.float32)
    out_glob[glob_of_new] = out_new
    return np.ascontiguousarray(out_glob[:N][:, np.argsort(p2)])


# revision 26
# speedup vs baseline: 1.6591x; 1.6591x over previous
"""2-layer GATv2 over 50k nodes / 1.6M edges on 8 trn2 NeuronCores.

Strategy (self-contained; shapes hardcoded for this problem):
  - Node-parallel dst sharding: nodes are degree-sorted and dealt round-robin
    to 8 cores (balanced slot counts); each core owns 6272 dst nodes.
  - Per-shard GEMMs only (no redundant full-table work); the per-shard xl
    tables are AllGathered on-device so each core can gather arbitrary
    source rows. Host->device traffic is just the core's own x shard (fp16),
    packed u16 slot tables, and small weights.
  - Per dst node, incoming edges live in up to D=64 "slots" (max degree 61);
    per-128-node-tile slot count Dt comes from the degree sort, cutting padded
    work from 64 to ~avg-degree slots. Slot tables ship as packed u16
    [128, sum(Dt)] and are widened to i32 on-device; the -1e30 pad-logit mask
    is derived on-device from idx >= NP.
  - att is folded into the weights on the host (u = att*z); leaky-relu logits
    are computed as sum_pos relu(u) - sum_neg relu(-u) via ACT with a host
    sign-permutation of feature columns; messages are recovered from u via a
    1/att columnwise multiply (exact up to fp rounding).
  - Gather of xl rows via indirect DMA (gpsimd SWDGE) with CCE add of the
    prefilled xr broadcast; pad slots masked via the on-device -1e30 offsets.
  - Layer-1 output h is transposed on PE into a resident fp16 SBUF tile and
    used directly as lhsT for the layer-2 per-shard GEMMs.
"""
import os
os.environ.setdefault("JAX_PLATFORMS", "cpu")
import sys
if "/opt/trn_rl_repo" not in sys.path:
    sys.path.insert(0, "/opt/trn_rl_repo")
import numpy as np
import concourse.bass as bass
import concourse.bacc as bacc
import concourse.mybir as mybir
import concourse.tile as tile
from concourse import bass_utils
from concourse.masks import make_identity

f32 = mybir.dt.float32
f16 = mybir.dt.float16
i32 = mybir.dt.int32
u16 = mybir.dt.uint16
AX = mybir.AxisListType
OP = mybir.AluOpType
AF = mybir.ActivationFunctionType

N = 50000
NCORES = 8
NP = 50176          # 8 * 6272, multiple of 1024
SH = NP // NCORES   # 6272 = 49 * 128
TPS = SH // 128     # 49 tiles per shard
F_IN = 256
H = 128
C = 64
DMAX = 64
NEG = 0.2
EPS = 1e-16

GATHER_MODE = "per_d"   # "per_d" | "multi"
VARIANT = "full"        # "full" | "nogather" | "noedge" (timing ablations)
BCAST_ADD = True        # single DVE broadcast add of xr (False: per-slot adds)
EDGE_SKIP = ()          # timing ablations: subset of
                        # {"acc", "logit", "exp", "trans"}

LAST_RESULT = None
LAST_RUN_WALL = None
_PROGRAM_CACHE = {}


def ts(i, s):
    return slice(i * s, (i + 1) * s)


def ceil4(v):
    return max(4, (int(v) + 3) // 4 * 4)


def build_program(Dts, Fp1, Fp2):
    key = (tuple(Dts), Fp1, Fp2, GATHER_MODE, VARIANT, BCAST_ADD,
           tuple(EDGE_SKIP))
    if key in _PROGRAM_CACHE:
        return _PROGRAM_CACHE[key]
    TOTD = sum(Dts)
    cums = [0]
    for d in Dts:
        cums.append(cums[-1] + d)

    nc = bacc.Bacc("TRN2", target_bir_lowering=False, debug=False,
                   enable_asserts=False, num_devices=NCORES)

    xTo = nc.dram_tensor("xTo", [F_IN, SH], f16, kind="ExternalInput")
    wl1 = nc.dram_tensor("wl1", [F_IN, H + 1], f16, kind="ExternalInput")
    wr1 = nc.dram_tensor("wr1", [F_IN, H + 1], f16, kind="ExternalInput")
    wl2 = nc.dram_tensor("wl2", [H, C + 1], f16, kind="ExternalInput")
    wr2 = nc.dram_tensor("wr2", [H, C + 1], f16, kind="ExternalInput")
    slotp = nc.dram_tensor("slotp", [128, TOTD], u16, kind="ExternalInput")
    rc1 = nc.dram_tensor("rc1", [128, H], f32, kind="ExternalInput")
    cb1 = nc.dram_tensor("cb1", [128, H], f32, kind="ExternalInput")
    rc2 = nc.dram_tensor("rc2", [128, C], f32, kind="ExternalInput")
    cb2 = nc.dram_tensor("cb2", [128, C], f32, kind="ExternalInput")
    outc = nc.dram_tensor("outc", [SH, C], f16, kind="ExternalOutput")

    xl1t = nc.dram_tensor("xl1t", [SH, H + 1], f16, kind="Internal")
    xl1f = nc.dram_tensor("xl1f", [NP, H + 1], f16, kind="Internal",
                          addr_space="Shared")
    xl2t = nc.dram_tensor("xl2t", [SH, C + 1], f16, kind="Internal")
    xl2f = nc.dram_tensor("xl2f", [NP, C + 1], f16, kind="Internal",
                          addr_space="Shared")

    with tile.TileContext(nc) as tc:
        with tc.tile_pool(name="pers", bufs=1) as pers:
            # persistent SBUF residents (span all phases)
            xr1_sb = pers.tile([128, TPS, H + 1], f16)
            hT_sb = pers.tile([128, SH], f16)
            xr2_sb = pers.tile([128, TPS, C + 1], f16)
            slot_i = pers.tile([128, TOTD], i32)
            moff_sb = pers.tile([128, TOTD], f32)
            rc1_t = pers.tile([128, H], f32)
            cb1_t = pers.tile([128, H], f32)
            rc2_t = pers.tile([128, C], f32)
            cb2_t = pers.tile([128, C], f32)
            ident = pers.tile([128, 128], f32)
            wl2_t = pers.tile([128, C + 1], f16)
            wr2_t = pers.tile([128, C + 1], f16)

            make_identity(nc, ident[:])
            nc.sync.dma_start(out=rc1_t[:], in_=rc1.ap())
            nc.sync.dma_start(out=cb1_t[:], in_=cb1.ap())
            nc.sync.dma_start(out=rc2_t[:], in_=rc2.ap())
            nc.sync.dma_start(out=cb2_t[:], in_=cb2.ap())
            nc.sync.dma_start(out=wl2_t[:], in_=wl2.ap())
            nc.sync.dma_start(out=wr2_t[:], in_=wr2.ap())

            # slot table: u16 load -> f32; mask from idx>=NP; clamp for the
            # gather (pads read row NP-1, masked to alpha=0 by moff)
            with tc.tile_pool(name="pslot", bufs=1) as pslot:
                slot_u = pslot.tile([128, TOTD], u16)
                nc.sync.dma_start(out=slot_u[:], in_=slotp.ap())
                slot_f = pslot.tile([128, TOTD], f32)
                nc.vector.tensor_copy(out=slot_f[:], in_=slot_u[:])
                nc.vector.tensor_scalar(out=moff_sb[:], in0=slot_f[:],
                                        scalar1=float(NP) - 0.5,
                                        scalar2=-1e30,
                                        op0=OP.is_ge, op1=OP.mult)
                nc.vector.tensor_scalar_min(slot_f[:], slot_f[:],
                                            float(NP - 1))
                nc.vector.tensor_copy(out=slot_i[:], in_=slot_f[:])

            # ---------------- Phase A: layer-1 GEMMs (own shard) ----------
            with (
                tc.tile_pool(name="paw", bufs=1) as pw,
                tc.tile_pool(name="pa", bufs=4) as pa,
                tc.tile_pool(name="pap", bufs=4, space="PSUM") as pp,
            ):
                wl_t = pw.tile([128, 2, H + 1], f16)
                wr_t = pw.tile([128, 2, H + 1], f16)
                for k in range(2):
                    nc.sync.dma_start(out=wl_t[:, k, :],
                                      in_=wl1.ap()[ts(k, 128), :])
                    nc.sync.dma_start(out=wr_t[:, k, :],
                                      in_=wr1.ap()[ts(k, 128), :])
                x_sb = pw.tile([128, 2, SH], f16)
                for k in range(2):
                    nc.sync.dma_start(out=x_sb[:, k, :],
                                      in_=xTo.ap()[ts(k, 128), :])
                for t in range(TPS):
                    psl_t = pp.tile([128, H + 1], f32, tag="psl")
                    for k in range(2):
                        nc.tensor.matmul(out=psl_t[:],
                                         lhsT=x_sb[:, k, ts(t, 128)],
                                         rhs=wl_t[:, k, :],
                                         start=(k == 0), stop=(k == 1))
                    o_t = pa.tile([128, H + 1], f16, tag="o")
                    nc.scalar.copy(out=o_t[:], in_=psl_t[:])
                    nc.sync.dma_start(out=xl1t.ap()[ts(t, 128), :], in_=o_t[:])
                    psr_t = pp.tile([128, H + 1], f32, tag="psr")
                    for k in range(2):
                        nc.tensor.matmul(out=psr_t[:],
                                         lhsT=x_sb[:, k, ts(t, 128)],
                                         rhs=wr_t[:, k, :],
                                         start=(k == 0), stop=(k == 1))
                    nc.scalar.copy(out=xr1_sb[:, t, :], in_=psr_t[:])

            # ---------------- AllGather layer-1 xl table ------------------
            nc.gpsimd.collective_compute(
                "AllGather", OP.bypass,
                replica_groups=[list(range(NCORES))],
                ins=[xl1t.ap()], outs=[xl1f.ap()])

            # ---------------- layer-1 edge phase --------------------------
            if VARIANT != "noedge":
                edge_phase(nc, tc, Dts, cums, Fp1, H, xl1f, xr1_sb, slot_i,
                           moff_sb, rc1_t, cb1_t, relu=True, out_dram=None,
                           hT_sb=hT_sb, ident=ident)
            else:
                nc.vector.memset(hT_sb[:], 0.0)

            # ---------------- Phase C: layer-2 GEMMs (own shard) ----------
            with (
                tc.tile_pool(name="pd", bufs=4) as pd,
                tc.tile_pool(name="pdp", bufs=4, space="PSUM") as pp2,
            ):
                for t in range(TPS):
                    ps2_t = pp2.tile([128, C + 1], f32, tag="ps2l")
                    nc.tensor.matmul(out=ps2_t[:], lhsT=hT_sb[:, ts(t, 128)],
                                     rhs=wl2_t[:], start=True, stop=True)
                    o2_t = pd.tile([128, C + 1], f16, tag="o2")
                    nc.scalar.copy(out=o2_t[:], in_=ps2_t[:])
                    nc.sync.dma_start(out=xl2t.ap()[ts(t, 128), :], in_=o2_t[:])
                    ps2r_t = pp2.tile([128, C + 1], f32, tag="ps2r")
                    nc.tensor.matmul(out=ps2r_t[:], lhsT=hT_sb[:, ts(t, 128)],
                                     rhs=wr2_t[:], start=True, stop=True)
                    nc.scalar.copy(out=xr2_sb[:, t, :], in_=ps2r_t[:])

            # ---------------- AllGather layer-2 xl table ------------------
            nc.gpsimd.collective_compute(
                "AllGather", OP.bypass,
                replica_groups=[list(range(NCORES))],
                ins=[xl2t.ap()], outs=[xl2f.ap()])

            # ---------------- layer-2 edge phase --------------------------
            if VARIANT != "noedge":
                edge_phase(nc, tc, Dts, cums, Fp2, C, xl2f, xr2_sb, slot_i,
                           moff_sb, rc2_t, cb2_t, relu=False, out_dram=outc,
                           hT_sb=None, ident=None)
            else:
                with tc.tile_pool(name="pdum", bufs=2) as pdum:
                    for t in range(TPS):
                        dt_ = pdum.tile([128, C], f16, tag="d")
                        nc.vector.tensor_copy(out=dt_[:],
                                              in_=xr2_sb[:, t, 0:C])
                        nc.sync.dma_start(out=outc.ap()[ts(t, 128), :],
                                          in_=dt_[:])

    nc.compile()
    _PROGRAM_CACHE[key] = nc
    return nc


def make_chunks(Dts, cap=224):
    """Group consecutive equal-Dt tiles into chunks of at most cap slots."""
    chunks = []
    t = 0
    while t < TPS:
        Dt = Dts[t]
        ch = 1
        while (t + ch < TPS and Dts[t + ch] == Dt and (ch + 1) * Dt <= cap):
            ch += 1
        chunks.append((t, ch, Dt))
        t += ch
    return chunks


def edge_phase(nc, tc, Dts, cums, Fp, F, xlf, xr_sb, slot_i, moff_sb,
               rc_t, cb_t, relu, out_dram, hT_sb, ident):
    TW = F + 1   # table width: F features + q (= row-sum) column
    with (
        tc.tile_pool(name=f"pz{F}", bufs=2) as pz,
        tc.tile_pool(name=f"pm{F}", bufs=3) as psm,
        tc.tile_pool(name=f"po{F}", bufs=2) as pout,
        tc.tile_pool(name=f"pp{F}", bufs=2, space="PSUM") as pps,
    ):
        for (t0, CH, Dt) in make_chunks(Dts):
            S = CH * Dt
            cu = cums[t0]
            off_t = moff_sb[:, cu:cu + S]
            xr_c = xr_sb[:, t0:t0 + CH, :]

            # z = gathered xl rows (bypass; pad idx clamped to NP-1), then
            # u = z + xr via in-place broadcast add.
            z_t = pz.tile([128, CH, Dt, TW], f16, tag="z")
            zf = z_t[:].rearrange("p c d w -> p (c d) w")
            if VARIANT == "nogather":
                nc.vector.tensor_copy(
                    out=z_t[:],
                    in_=xr_c[:, :, None, :].to_broadcast([128, CH, Dt, TW]))
            else:
                for c in range(CH):
                    cuc = cums[t0 + c]
                    for d in range(Dt):
                        nc.gpsimd.indirect_dma_start(
                            out=z_t[:, c, d, :], out_offset=None,
                            in_=xlf.ap(),
                            in_offset=bass.IndirectOffsetOnAxis(
                                ap=slot_i[:, cuc + d:cuc + d + 1], axis=0),
                            bounds_check=None, compute_op=OP.bypass)
            nc.vector.tensor_tensor(
                out=z_t[:], in0=z_t[:],
                in1=xr_c[:, :, None, :].to_broadcast([128, CH, Dt, TW]),
                op=OP.add)

            # logits. With u = xl+xr (in z) and the q column sigma = sum_f u:
            #   0.8*(sum_pos relu(u) - sum_neg relu(-u)) + 0.2*sigma
            #     = 0.6*sigma + 0.4*(sum_pos |u| - sum_neg |u|)
            # so two abs-reduces replace the ACT relus entirely.
            e_t = psm.tile([128, CH, Dt], f32, tag="e")
            ef = e_t[:].rearrange("p c d -> p (c d)")
            ep_t = psm.tile([128, S], f32, tag="ep")
            nc.vector.tensor_reduce(out=ep_t[:], in_=zf[:, :, 0:Fp],
                                    axis=AX.X, op=OP.add,
                                    apply_absolute_value=True)
            en_t = psm.tile([128, S], f32, tag="en")
            nc.vector.tensor_reduce(out=en_t[:], in_=zf[:, :, Fp:F],
                                    axis=AX.X, op=OP.add,
                                    apply_absolute_value=True)
            nc.vector.scalar_tensor_tensor(out=ef, in0=en_t[:],
                                           scalar=-1.0, in1=ep_t[:],
                                           op0=OP.mult, op1=OP.add)
            nc.vector.scalar_tensor_tensor(out=ef, in0=zf[:, :, F],
                                           scalar=1.5, in1=ef,
                                           op0=OP.mult, op1=OP.add)
            nc.vector.scalar_tensor_tensor(out=ef, in0=ef,
                                           scalar=0.4, in1=off_t,
                                           op0=OP.mult, op1=OP.add)
            # softmax over slots of each dst node (c-row)
            mneg_t = psm.tile([128, CH], f32, tag="mneg")
            nc.vector.tensor_reduce(out=mneg_t[:], in_=e_t[:], axis=AX.X,
                                    op=OP.max, negate=True)
            nc.vector.tensor_scalar_min(mneg_t[:], mneg_t[:], 1e29)
            nc.vector.tensor_tensor(
                out=e_t[:], in0=e_t[:],
                in1=mneg_t[:, :, None].to_broadcast([128, CH, Dt]),
                op=OP.add)
            a_t = psm.tile([128, CH, Dt], f32, tag="a")
            nc.scalar.activation(out=a_t[:].rearrange("p c d -> p (c d)"),
                                 in_=ef, func=AF.Exp)
            s_t = psm.tile([128, CH], f32, tag="s")
            nc.vector.tensor_reduce(out=s_t[:], in_=a_t[:], axis=AX.X,
                                    op=OP.add)
            nc.vector.tensor_scalar_add(s_t[:], s_t[:], EPS)
            r_t = psm.tile([128, CH], f32, tag="r")
            nc.vector.reciprocal(out=r_t[:], in_=s_t[:])
            al_t = psm.tile([128, CH, Dt], f32, tag="al")
            nc.vector.tensor_tensor(
                out=al_t[:], in0=a_t[:],
                in1=r_t[:, :, None].to_broadcast([128, CH, Dt]), op=OP.mult)

            # message aggregation: msg = sum_d alpha_d * u_d - (sum alpha)*xr
            # (z holds u = xr+g). In-place alpha multiply + one reduce over
            # the slot axis via a transposed AP view.
            nc.vector.tensor_tensor(
                out=z_t[:, :, :, 0:F], in0=z_t[:, :, :, 0:F],
                in1=al_t[:, :, :, None].to_broadcast([128, CH, Dt, F]),
                op=OP.mult)
            acc_t = pout.tile([128, CH, F], f32, tag="acc")
            nc.vector.tensor_reduce(
                out=acc_t[:], in_=z_t[:, :, :, 0:F].transpose([0, 1, 3, 2]),
                axis=AX.X, op=OP.add)
            saneg_t = psm.tile([128, CH], f32, tag="saneg")
            nc.vector.tensor_reduce(out=saneg_t[:], in_=al_t[:],
                                    axis=AX.X, op=OP.add, negate=True)
            hh_t = pout.tile([128, CH, F], f32, tag="hh")
            nc.vector.tensor_tensor(
                out=hh_t[:], in0=xr_c[:, :, 0:F],
                in1=saneg_t[:, :, None].to_broadcast([128, CH, F]),
                op=OP.mult)
            nc.vector.tensor_tensor(out=hh_t[:], in0=hh_t[:], in1=acc_t[:],
                                    op=OP.add)
            nc.vector.tensor_tensor(
                out=hh_t[:], in0=hh_t[:],
                in1=rc_t[:, None, :].to_broadcast([128, CH, F]), op=OP.mult)
            nc.vector.tensor_tensor(
                out=hh_t[:], in0=hh_t[:],
                in1=cb_t[:, None, :].to_broadcast([128, CH, F]), op=OP.add)
            if relu:
                nc.vector.tensor_scalar_max(hh_t[:], hh_t[:], 0.0)
                for c in range(CH):
                    pt_t = pps.tile([128, 128], f32, tag="pt")
                    nc.tensor.transpose(out=pt_t[:], in_=hh_t[:, c, :],
                                        identity=ident[:])
                    nc.scalar.copy(out=hT_sb[:, ts(t0 + c, 128)], in_=pt_t[:])
            else:
                ho_t = pout.tile([128, CH, F], f16, tag="ho")
                nc.vector.tensor_copy(out=ho_t[:], in_=hh_t[:])
                for c in range(CH):
                    nc.sync.dma_start(
                        out=out_dram.ap()[ts(t0 + c, 128), :],
                        in_=ho_t[:, c, :])


def prepare_host(x, edge_index, Wl1, Wr1, att1, b1, Wl2, Wr2, att2, b2):
    src = np.asarray(edge_index[0], dtype=np.int64)
    dst = np.asarray(edge_index[1], dtype=np.int64)
    x = np.asarray(x, dtype=np.float32)

    deg = np.bincount(dst, minlength=NP).astype(np.int64)
    assert deg.max() <= DMAX, f"max degree {deg.max()} > {DMAX}"
    order = np.argsort(-deg, kind="stable")
    q = np.arange(NP)
    new_of = np.empty(NP, dtype=np.int64)
    new_of[order] = (q % NCORES) * SH + q // NCORES
    glob_of_new = np.empty(NP, dtype=np.int64)
    glob_of_new[new_of] = np.arange(NP)

    # slot tables (values are NEW ids; rows ordered by NEW id)
    eorder = np.argsort(dst, kind="stable")
    s_src = src[eorder]
    s_dst = dst[eorder]
    starts = np.zeros(NP, dtype=np.int64)
    starts[1:] = np.cumsum(deg)[:-1]
    pos = np.arange(len(s_dst)) - starts[s_dst]
    # pads point at idx=NP: skipped by the gather bounds check, masked by the
    # on-device idx>=NP -> -1e30 logit offset
    slot_g = np.full((NP, DMAX), NP, dtype=np.int32)
    slot_g[s_dst, pos] = new_of[s_src].astype(np.int32)
    slot_new = slot_g[glob_of_new]

    deg_sorted = deg[order]
    Dts = tuple(ceil4(max(deg_sorted[1024 * t], 1)) for t in range(TPS))

    att1 = np.asarray(att1, np.float32)
    att2 = np.asarray(att2, np.float32)
    assert np.abs(att1).min() > 1e-8 and np.abs(att2).min() > 1e-8
    p1 = np.argsort(att1 < 0, kind="stable")
    Fp1 = int((att1 >= 0).sum())
    p2 = np.argsort(att2 < 0, kind="stable")
    Fp2 = int((att2 >= 0).sum())
    # fold att into weight columns, sign-permute, and append a row-sum
    # column (the q/sigma channel: sum_f u = x @ wsum)
    def fold(W, att, perm, rowperm=None):
        Wa = (np.asarray(W, np.float32) * att)
        if rowperm is not None:
            Wa = Wa[rowperm, :]
        Wp = Wa[:, perm]
        return np.ascontiguousarray(
            np.concatenate([Wp, Wp.sum(1, keepdims=True)], axis=1),
            np.float16)

    Wl1a = fold(Wl1, att1, p1)
    Wr1a = fold(Wr1, att1, p1)
    Wl2a = fold(Wl2, att2, p2, rowperm=p1)
    Wr2a = fold(Wr2, att2, p2, rowperm=p1)
    rc1_row = (1.0 / att1[p1]).astype(np.float32)
    rc2_row = (1.0 / att2[p2]).astype(np.float32)
    b1_row = np.asarray(b1, np.float32)[p1]
    b2_row = np.asarray(b2, np.float32)[p2]

    xp = np.zeros((NP, F_IN), np.float32)
    xp[:N] = x
    xT_perm = xp[glob_of_new].T.astype(np.float16)

    rep = lambda row: np.ascontiguousarray(
        np.tile(row[None, :], (128, 1)).astype(np.float32))
    common = dict(
        wl1=Wl1a, wr1=Wr1a, wl2=Wl2a, wr2=Wr2a,
        rc1=rep(rc1_row), cb1=rep(b1_row), rc2=rep(rc2_row), cb2=rep(b2_row))
    in_maps = []
    for c in range(NCORES):
        m = dict(common)
        m["xTo"] = np.ascontiguousarray(xT_perm[:, ts(c, SH)])
        sl = slot_new[ts(c, SH)]
        m["slotp"] = np.ascontiguousarray(np.concatenate(
            [sl[ts(t, 128), 0:Dts[t]] for t in range(TPS)],
            axis=1).astype(np.uint16))
        in_maps.append(m)
    return in_maps, Dts, Fp1, Fp2, glob_of_new, p2


def kernel(**inputs):
    global LAST_RESULT, LAST_RUN_WALL
    import time as _time
    in_maps, Dts, Fp1, Fp2, glob_of_new, p2 = prepare_host(**inputs)
    nc = build_program(Dts, Fp1, Fp2)
    _t0 = _time.time()
    res = bass_utils.run_bass_kernel_spmd(nc, in_maps,
                                          core_ids=list(range(NCORES)))
    LAST_RUN_WALL = _time.time() - _t0
    LAST_RESULT = res
    out_new = np.concatenate([res.results[c]["outc"] for c in range(NCORES)],
                             axis=0).astype(np.float32)
    out_glob = np.empty((NP, C), np.float32)
    out_glob[glob_of_new] = out_new
    return np.ascontiguousarray(out_glob[:N][:, np.argsort(p2)])
